# revision 12
# baseline (speedup 1.0000x reference)
"""Trainium2 Bass kernel: 2-layer LSTM over word embeddings + dense head.

Model (per reference):
  x = emb[tokens]                      # [B=64, S=512, E=300]
  h1 = LSTM_256(x); h2 = LSTM_256(h1)  # gates f,i,c(g),o ; combined z @ W
  out = sigmoid(relu(h2[:, -1] @ Wd + bd) @ Wout + bout)   # [B, 1]

Sharding: data-parallel over batch, 8 cores x 8 rows each; weights and the
embedding table replicated.

Device-side layout is feature-major ("transposed"): activations live as
[feature -> partition, batch -> free] so the per-step gate math runs on
128-partition tiles with batch=8 in the free dimension:
  - Embedding lookup: indirect-DMA gather (token-major) -> DRAM stage ->
    XBAR DMA-transpose into feature-major SBUF chunks (E padded to 384).
  - Input projections (x @ W1x, h1 @ W2x + biases) are batched over chunks
    of timesteps on the PE; bias is added during the PSUM->SBUF copy.
  - The serial recurrence (h_{t-1} @ Whh) keeps weights stationary
    (bf16 -> fast weight load) and streams batch=8; the precomputed input
    part is accumulated into the same PSUM tile with an identity matmul.
  - Layer 2 runs one chunk of steps behind layer 1 so each layer's
    ACT/DVE elementwise chain hides under the other layer's PE block.
  - PE runs bf16 (fp32 PSUM accumulate); cell state and nonlinearities fp32.
"""

import numpy as np
import ml_dtypes

BF16 = ml_dtypes.bfloat16

# Problem constants (hardcoded; kernel.py must be self-contained).
V, E, E_PAD = 50000, 300, 384
U = 256          # hidden units per LSTM layer
G4 = 4 * U       # 4 gates stacked: f, i, o, g
DNS = 128        # dense units
B, S = 64, 512
NCORES = 8
BL = B // NCORES  # batch rows per core = 8

_BUILD_CACHE = {}


def _build(S_, CH):
    """Build the Bass program (shared SPMD across all cores)."""
    import concourse.bass as bass
    import concourse.bacc as bacc
    import concourse.mybir as mybir
    from concourse.tile import TileContext
    from concourse.bass import ts

    AF = mybir.ActivationFunctionType
    dt = mybir.dt
    f32, bf16, i32 = dt.float32, dt.bfloat16, dt.int32

    T = S_ * BL            # tokens per core
    NCH = S_ // CH         # number of step chunks
    assert S_ % CH == 0 and T % 128 == 0

    nc = bacc.Bacc("TRN2", target_bir_lowering=False)

    # ---- DRAM I/O ----
    emb_d = nc.dram_tensor("emb", [V, E_PAD], bf16, kind="ExternalInput")
    tok_d = nc.dram_tensor("tok", [T, 1], i32, kind="ExternalInput")
    w1x_d = nc.dram_tensor("w1x", [128, 3 * G4], bf16, kind="ExternalInput")
    w1h_d = nc.dram_tensor("w1h", [128, 2 * G4], bf16, kind="ExternalInput")
    w2x_d = nc.dram_tensor("w2x", [128, 2 * G4], bf16, kind="ExternalInput")
    w2h_d = nc.dram_tensor("w2h", [128, 2 * G4], bf16, kind="ExternalInput")
    b1_d = nc.dram_tensor("b1", [1, G4], bf16, kind="ExternalInput")
    b2_d = nc.dram_tensor("b2", [1, G4], bf16, kind="ExternalInput")
    wd_d = nc.dram_tensor("wd", [128, 2 * DNS], bf16, kind="ExternalInput")
    bd_d = nc.dram_tensor("bd", [1, DNS], bf16, kind="ExternalInput")
    wo_d = nc.dram_tensor("wo", [128, 1], bf16, kind="ExternalInput")
    bo_d = nc.dram_tensor("bo", [1, 1], bf16, kind="ExternalInput")
    ident_d = nc.dram_tensor("ident", [128, 128], bf16, kind="ExternalInput")
    out_d = nc.dram_tensor("out", [1, BL], f32, kind="ExternalOutput")

    with TileContext(nc) as tc:
        from contextlib import ExitStack

        with ExitStack() as ex:
            stat = ex.enter_context(tc.tile_pool(name="static", bufs=1))
            dram = ex.enter_context(tc.tile_pool(name="dram", bufs=1, space="DRAM"))
            tokp = ex.enter_context(tc.tile_pool(name="tokp", bufs=1))
            gthp = ex.enter_context(tc.tile_pool(name="gthp", bufs=1))
            xb1p = ex.enter_context(tc.tile_pool(name="xb1p", bufs=2))
            xb2p = ex.enter_context(tc.tile_pool(name="xb2p", bufs=2))
            actp = ex.enter_context(tc.tile_pool(name="actp", bufs=4))
            tmpp = ex.enter_context(tc.tile_pool(name="tmpp", bufs=8))
            pstp = ex.enter_context(tc.tile_pool(name="pstp", bufs=2, space="PSUM"))
            ps1 = ex.enter_context(tc.tile_pool(name="ps1", bufs=2, space="PSUM"))
            ps2 = ex.enter_context(tc.tile_pool(name="ps2", bufs=2, space="PSUM"))
            psx = ex.enter_context(tc.tile_pool(name="psx", bufs=2, space="PSUM"))

            # ---- static SBUF tensors ----
            w1x = stat.tile([128, 3 * G4], bf16, name="w1x_sb")
            w1h = stat.tile([128, 2 * G4], bf16, name="w1h_sb")
            w2x = stat.tile([128, 2 * G4], bf16, name="w2x_sb")
            w2h = stat.tile([128, 2 * G4], bf16, name="w2h_sb")
            b1 = stat.tile([1, G4], bf16, name="b1_sb")
            b2 = stat.tile([1, G4], bf16, name="b2_sb")
            ones = stat.tile([1, 512], bf16, name="ones_sb")
            wd = stat.tile([128, 2 * DNS], bf16, name="wd_sb")
            bd = stat.tile([1, DNS], bf16, name="bd_sb")
            wo = stat.tile([128, 1], bf16, name="wo_sb")
            bo = stat.tile([1, 1], bf16, name="bo_sb")
            ident = stat.tile([128, 128], bf16, name="ident_sb")
            xt = [stat.tile([128, T], bf16, name=f"xt{k}_sb") for k in range(3)]
            H1 = stat.tile([128, 16 * S_], bf16, name="h1_sb")
            H2 = stat.tile([128, 16 * S_], bf16, name="h2_sb")
            c1 = stat.tile([128, 16], f32, name="c1_sb")
            c2 = stat.tile([128, 16], f32, name="c2_sb")
            zh = stat.tile([128, 16], bf16, name="zh_sb")
            dns = stat.tile([128, BL], bf16, name="dns_sb")
            osb = stat.tile([1, BL], f32, name="o_sb")

            # ---- load weights / constants ----
            for sb_t, dr_t in [
                (w1x, w1x_d), (w1h, w1h_d), (w2x, w2x_d), (w2h, w2h_d),
                (b1, b1_d), (b2, b2_d), (wd, wd_d), (bd, bd_d),
                (wo, wo_d), (bo, bo_d), (ident, ident_d),
            ]:
                nc.sync.dma_start(sb_t[:], dr_t[:])
            nc.gpsimd.memset(ones[:], 1.0)
            nc.gpsimd.memset(c1[:], 0.0)
            nc.gpsimd.memset(c2[:], 0.0)
            nc.gpsimd.memset(zh[:], 0.0)

            # ---- embedding gather (token-major) + transpose to feature-major
            # One token-index load, 32 indirect gathers into column blocks of
            # a single wide SBUF buffer, then per-tile SBUF->SBUF XBAR
            # transposes into xt[k][f, token].
            nt = T // 128
            tka = tokp.tile([128, nt], i32, name="tka")
            nc.sync.dma_start(
                tka[:].rearrange("p (i x) -> p i x", x=1),
                tok_d[:].rearrange("(i p) x -> p i x", p=128))
            gall = gthp.tile([128, nt * E_PAD], bf16, name="gall")
            for i in range(nt):
                nc.gpsimd.indirect_dma_start(
                    out=gall[:, i * E_PAD:(i + 1) * E_PAD],
                    out_offset=None,
                    in_=emb_d[:],
                    in_offset=bass.IndirectOffsetOnAxis(ap=tka[:, i:i + 1], axis=0),
                )
                for k in range(3):
                    pst = pstp.tile([128, 128], bf16, name="pst")
                    nc.tensor.transpose(
                        pst[:],
                        gall[:, i * E_PAD + k * 128: i * E_PAD + (k + 1) * 128],
                        ident[:],
                    )
                    nc.vector.tensor_copy(xt[k][:, ts(i, 128)], pst[:])

            # ---- batched input projections for a chunk of CH steps ----
            def xpre_chunk(layer, c):
                """Returns SBUF tile [128, 8*CH*8] bf16, laid out j-major:
                col = j*(CH*8) + t_local*8 + b, partition = gate unit % 128,
                j = gate unit // 128."""
                if layer == 1:
                    pool, wmat, nk, bias = xb1p, w1x, 3, b1
                    rhs_k = lambda k: xt[k][:, c * CH * 8:(c + 1) * CH * 8]
                else:
                    pool, wmat, nk, bias = xb2p, w2x, 2, b2
                    h1r = H1[:].rearrange("p (t r) -> p t r", r=16)
                    rhs_k = lambda k: h1r[:, c * CH:(c + 1) * CH,
                                          k * 8:(k + 1) * 8]
                buf = pool.tile([128, 8 * CH * 8], bf16, name=f"xb{layer}")
                for j in range(8):
                    ps = psx.tile([128, CH * 8], f32, name="psx")
                    for k in range(nk):
                        nc.tensor.matmul(
                            ps[:],
                            lhsT=wmat[:, k * G4 + j * 128: k * G4 + (j + 1) * 128],
                            rhs=rhs_k(k),
                            start=(k == 0),
                            stop=False,
                        )
                    # bias: rank-1 update  ps[p, n] += bias[128j + p] * 1
                    nc.tensor.matmul(
                        ps[:],
                        lhsT=bias[0:1, j * 128:(j + 1) * 128],
                        rhs=ones[0:1, 0:CH * 8],
                        start=False, stop=True,
                    )
                    nc.vector.tensor_copy(
                        buf[:, j * CH * 8:(j + 1) * CH * 8], ps[:])
                return buf

            # ---- one LSTM step (feature-major) ----
            def lstm_step(pool, wh, xbuf, tl, t, H, c_sb):
                ps = pool.tile([128, 64], f32, name="psr")
                # input-projection part: psum[:, 8j+b] = xbuf[p, j, tl, b]
                xr = xbuf[:].rearrange("p (j r) -> p j r", j=8)
                nc.tensor.matmul(
                    ps[:], lhsT=ident[:],
                    rhs=xr[:, :, tl * 8:(tl + 1) * 8],
                    start=True, stop=False, skip_group_check=True,
                )
                # recurrent part: 16 stationary weight tiles, batch moving
                for j in range(8):
                    for k in range(2):
                        hprev = (zh[:, k * 8:(k + 1) * 8] if t == 0 else
                                 H[:, (t - 1) * 16 + k * 8:(t - 1) * 16 + (k + 1) * 8])
                        nc.tensor.matmul(
                            ps[:, j * 8:(j + 1) * 8],
                            lhsT=wh[:, k * G4 + j * 128: k * G4 + (j + 1) * 128],
                            rhs=hprev,
                            start=False, stop=(k == 1), skip_group_check=True,
                        )
                # gates: cols [f 0:16 | i 16:32 | o 32:48 | g 48:64]
                acts = actp.tile([128, 64], f32, name="acts")
                nc.scalar.activation(acts[:, 0:48], ps[:, 0:48], AF.Sigmoid)
                nc.scalar.activation(acts[:, 48:64], ps[:, 48:64], AF.Tanh)
                t1 = tmpp.tile([128, 16], f32, name="t1")
                t2 = tmpp.tile([128, 16], f32, name="t2")
                nc.vector.tensor_mul(t1[:], acts[:, 0:16], c_sb[:])     # f*c
                nc.vector.tensor_mul(t2[:], acts[:, 16:32], acts[:, 48:64])  # i*g
                nc.vector.tensor_add(c_sb[:], t1[:], t2[:])
                th = tmpp.tile([128, 16], f32, name="th")
                nc.scalar.activation(th[:], c_sb[:], AF.Tanh)
                nc.vector.tensor_mul(H[:, t * 16:(t + 1) * 16], acts[:, 32:48], th[:])

            # ---- main pipeline: L1 chunk c runs with L2 chunk c-1 ----
            xb1 = xpre_chunk(1, 0)
            xb2 = None
            for c in range(NCH):
                for tl in range(CH):
                    t = c * CH + tl
                    lstm_step(ps1, w1h, xb1, tl, t, H1, c1)
                    if c >= 1:
                        lstm_step(ps2, w2h, xb2, tl, t - CH, H2, c2)
                if c + 1 < NCH:
                    xb1 = xpre_chunk(1, c + 1)
                xb2 = xpre_chunk(2, c)
            for tl in range(CH):  # layer-2 tail chunk
                lstm_step(ps2, w2h, xb2, tl, S_ - CH + tl, H2, c2)

            # ---- dense head on final h2 ----
            psd = ps1.tile([128, 64], f32, name="psr")
            for k in range(2):
                nc.tensor.matmul(
                    psd[:, 0:BL],
                    lhsT=wd[:, k * DNS:(k + 1) * DNS],
                    rhs=H2[:, (S_ - 1) * 16 + k * 8:(S_ - 1) * 16 + (k + 1) * 8],
                    start=(k == 0), stop=False,
                )
            nc.tensor.matmul(psd[:, 0:BL], lhsT=bd[0:1, :], rhs=ones[0:1, 0:BL],
                             start=False, stop=True, skip_group_check=True)
            nc.scalar.activation(dns[:], psd[:, 0:BL], AF.Relu)
            pso = ps2.tile([128, 64], f32, name="psr")
            nc.tensor.matmul(pso[0:1, 0:BL], lhsT=wo[:, 0:1], rhs=dns[:],
                             start=True, stop=False, skip_group_check=True)
            nc.tensor.matmul(pso[0:1, 0:BL], lhsT=bo[0:1, 0:1], rhs=ones[0:1, 0:BL],
                             start=False, stop=True, skip_group_check=True)
            nc.scalar.activation(osb[:], pso[0:1, 0:BL], AF.Sigmoid)
            nc.sync.dma_start(out_d[:], osb[:])

    nc.compile()
    return nc


def _pack_weights(inputs):
    """Host-side packing into the device layouts (gate order f, i, o, g)."""
    f32 = np.float32

    def gates(prefix):
        return [np.asarray(inputs[prefix + g], f32) for g in ("f", "i", "o", "c")]

    W1 = gates("W1")   # each [E+U, U]
    W2 = gates("W2")   # each [2U, U]
    b1 = np.concatenate([np.asarray(inputs["b1" + g], f32) for g in ("f", "i", "o", "c")])
    b2 = np.concatenate([np.asarray(inputs["b2" + g], f32) for g in ("f", "i", "o", "c")])

    w1x_full = np.concatenate([w[:E] for w in W1], axis=1)        # [300, 1024]
    w1x_full = np.concatenate(
        [w1x_full, np.zeros((E_PAD - E, G4), f32)], axis=0)       # [384, 1024]
    w1x = np.concatenate([w1x_full[k * 128:(k + 1) * 128] for k in range(3)],
                         axis=1).astype(BF16)                     # [128, 3072]
    w1h_full = np.concatenate([w[E:] for w in W1], axis=1)        # [256, 1024]
    w1h = np.concatenate([w1h_full[k * 128:(k + 1) * 128] for k in range(2)],
                         axis=1).astype(BF16)                     # [128, 2048]
    w2x_full = np.concatenate([w[:U] for w in W2], axis=1)
    w2x = np.concatenate([w2x_full[k * 128:(k + 1) * 128] for k in range(2)],
                         axis=1).astype(BF16)
    w2h_full = np.concatenate([w[U:] for w in W2], axis=1)
    w2h = np.concatenate([w2h_full[k * 128:(k + 1) * 128] for k in range(2)],
                         axis=1).astype(BF16)

    wd_full = np.asarray(inputs["Wd"], f32)                       # [256, 128]
    wd = np.concatenate([wd_full[k * 128:(k + 1) * 128] for k in range(2)],
                        axis=1).astype(BF16)                      # [128, 256]
    pack = {
        "w1x": w1x, "w1h": w1h, "w2x": w2x, "w2h": w2h,
        "b1": b1.astype(BF16).reshape(1, G4),
        "b2": b2.astype(BF16).reshape(1, G4),
        "wd": wd,
        "bd": np.asarray(inputs["bd"], f32).astype(BF16).reshape(1, DNS),
        "wo": np.asarray(inputs["Wout"], f32).astype(BF16).reshape(128, 1),
        "bo": np.asarray(inputs["bout"], f32).astype(BF16).reshape(1, 1),
        "ident": np.eye(128, dtype=BF16),
    }
    emb = np.asarray(inputs["emb"], f32)
    emb_pad = np.zeros((V, E_PAD), BF16)
    emb_pad[:, :E] = emb.astype(BF16)
    pack["emb"] = emb_pad
    return pack


def kernel(**inputs):
    from concourse.bass_utils import run_bass_kernel_spmd

    tokens = np.asarray(inputs["tokens"])
    S_ = tokens.shape[1]
    CH = 32 if S_ % 32 == 0 else 16
    key = (S_, CH)
    if key not in _BUILD_CACHE:
        _BUILD_CACHE[key] = _build(S_, CH)
    nc = _BUILD_CACHE[key]

    pack = _pack_weights(inputs)
    in_maps = []
    for core in range(NCORES):
        tok = tokens[core * BL:(core + 1) * BL].astype(np.int32)  # [8, S]
        tok = np.ascontiguousarray(tok.T).reshape(-1, 1)          # f = t*8 + b
        in_maps.append({**pack, "tok": tok})

    res = run_bass_kernel_spmd(nc, in_maps, core_ids=list(range(NCORES)))
    global _LAST_RESULTS
    _LAST_RESULTS = res
    out = np.concatenate(
        [r["out"].reshape(BL, 1) for r in res.results], axis=0
    ).astype(np.float32)
    return out


_LAST_RESULTS = None


if __name__ == "__main__":
    import reference

    inputs = {k: np.asarray(v) for k, v in reference.setup_inputs().items()}
    got = kernel(**inputs)
    want = np.asarray(reference.reference(**reference.setup_inputs()))
    err = np.abs(got - want).max() / max(np.abs(want).max(), 1e-9)
    print("max rel err:", err)
